# revision 46
# baseline (speedup 1.0000x reference)
"""Trainium2 Bass kernel for nn_Encoder (sliding-window MLP + synaptic conv).

Computation (per timestep t of T_data):
  syn_e[t] = sum(S_e[t, :]);  syn_i[t] = sum(S_i[t, :])
  syn_out[t, s] = sum_k e_kern[s, k] * syn_e[t-k] + i_kern[s, k] * syn_i[t-k]
  Vw[t, :] = V[t-199 : t+200]   (zero padded)
  h = lrelu(Vw @ W1.T + b1); h = lrelu(h @ W2.T + b2); h = lrelu(h @ W3.T + b3)
  out[t, :] = tanh(h @ W4.T + b4 + syn_out[t, :])

Strategy: data-parallel over T across 8 NeuronCores; each core gets its T/8
slice plus a 199-row halo of S_e/S_i and a V halo (host zero-pads edges).
Per core:
  - S_e/S_i stream in fp8e4 (4 row-tiles per DMA into [128,4,C] tiles),
    GpSimd folds column halves fp8->bf16, VectorE free-axis reduces the
    bf16 fold at 2x rate into fp32 accumulators; accumulators are
    PE-transposed and stored to a DRAM scratch vector in bf16.
  - MLP in fp8e4 with DoubleRow matmuls (2 K-chunks of 128 per PE
    instruction): weights live as [128, 4, 512] tiles (16x scaled),
    activations as [128, 4, 512] fp8 tiles whose planes are hid chunks.
    The V sliding window is materialized as a [128, 4, W] Hankel tile via
    a single 3D overlapped DMA (plane kc = window offset 128*kc).
  - Blocks of 512 timesteps are processed in groups of 4, sub-paired
    (a,b)/(c,d): each DoubleRow stationary serves 2 consecutive matmuls
    (hides the slow DR LDWEIGHTS) and each layer's PSUM evictions retire
    under the other sub-pair's matmuls.
  - ScalarE evicts PSUM with fused lrelu (scale=1/16 undoes the weight
    scaling, bias folded in); layer-4 ff evicts via VectorE copy.
  - Conv runs one group behind the MLP (scratch operands guaranteed
    ready): e/i kernels matmul into col-grouped PSUM [0:20]/[32:52] of
    one bank, then ff + e + i are summed by 2 VectorE adds and ScalarE
    applies tanh with b4 as the per-partition activation bias.
"""

import os
from contextlib import ExitStack

import ml_dtypes
import numpy as np

import concourse.bass as bass
import concourse.mybir as mybir
import concourse.tile as tile
from concourse import bacc
from concourse.bass_utils import run_bass_kernel_spmd
from concourse.masks import make_identity
from concourse.tile_rust import add_dep_helper

BF16 = ml_dtypes.bfloat16
E4M3 = ml_dtypes.float8_e4m3fn
FP32 = mybir.dt.float32
BF = mybir.dt.bfloat16
F8 = mybir.dt.float8e4

T_NO = 200
WIN = 2 * T_NO - 1  # 399
N_CORES = 8
BLK = 512  # timesteps per block (one PSUM bank of fp32)
GRP = 4  # blocks per group
RED_K = 4  # S_e/S_i row-tiles per DMA/fold/reduce step
WS = 16.0  # fp8 weight scale

LAST = {}  # exec_time_ns / trace info from the most recent run (for test harness)


def _ceil_div(a, b):
    return -(-a // b)


def _build(T_PAD, SE_ROWS, E_COLS, I_COLS, HID, SUB):
    """Build the per-core Bass program (identical on all 8 cores)."""
    R_TILES = _ceil_div(SE_ROWS, 128)
    R_STEPS = _ceil_div(R_TILES, RED_K)
    R_ROWS = R_STEPS * RED_K * 128  # host pads S_e/S_i to this many rows
    SCR_LEN = R_STEPS * RED_K * 128 + 128
    V_LEN = T_PAD + WIN - 1 + 128
    NB = _ceil_div(T_PAD, BLK)
    NKH = 4  # K chunks of both WIN (399) and HID (500), 128-padded
    HID_P = 512

    nc = bacc.Bacc(
        "TRN2", target_bir_lowering=False, debug=False, num_devices=N_CORES
    )

    se_h = nc.dram_tensor("se", [R_ROWS, E_COLS], F8, kind="ExternalInput")
    si_h = nc.dram_tensor("si", [R_ROWS, I_COLS], F8, kind="ExternalInput")
    v_h = nc.dram_tensor("v", [V_LEN], F8, kind="ExternalInput")
    w1_h = nc.dram_tensor("w1", [128, NKH * HID_P], F8, kind="ExternalInput")
    w2_h = nc.dram_tensor("w2", [128, NKH * HID_P], F8, kind="ExternalInput")
    w3_h = nc.dram_tensor("w3", [128, NKH * HID_P], F8, kind="ExternalInput")
    w4_h = nc.dram_tensor("w4", [128, NKH * 32], F8, kind="ExternalInput")
    bpk_h = nc.dram_tensor("bpk", [128, 3 * NKH], FP32, kind="ExternalInput")
    b4_h = nc.dram_tensor("b4", [SUB], FP32, kind="ExternalInput")
    spk_h = nc.dram_tensor("spk", [128, 4 * SUB], BF, kind="ExternalInput")
    out_h = nc.dram_tensor("out", [SUB, T_PAD], FP32, kind="ExternalOutput")

    sse_h = nc.dram_tensor("sse_scratch", [SCR_LEN], BF)
    ssi_h = nc.dram_tensor("ssi_scratch", [SCR_LEN], BF)

    # group structure: a short first group (2 blocks) so its scratch window
    # is ready early, then groups of 4
    blocks = []
    for b in range(NB):
        blocks.append((BLK * b, min(BLK, T_PAD - BLK * b)))
    starts = [0] + list(range(min(2, NB), NB, GRP))
    groups = []
    for si in range(len(starts)):
        g0 = starts[si]
        g1 = starts[si + 1] if si + 1 < len(starts) else NB
        blks = []
        off = 0
        for b in range(g0, g1):
            bt0, nt = blocks[b]
            blks.append((bt0, nt, off))
            off += nt
        groups.append((BLK * g0, blks, off))
    NG = len(groups)
    SY_W = 128  # extra hankel cols for the 2nd conv K chunk

    # reduce steps needed before the conv of group g can load its hankel
    needs = []
    for gi, (t0g, blks, tot) in enumerate(groups):
        if gi == NG - 1:
            needs.append(R_STEPS)
        else:
            needs.append(min(R_STEPS,
                             _ceil_div(_ceil_div(t0g + tot + T_NO - 1, 128),
                                       RED_K)))

    with tile.TileContext(nc) as tc, ExitStack() as ctx:
        cpool = ctx.enter_context(tc.tile_pool(name="consts", bufs=1))
        sepool = ctx.enter_context(tc.tile_pool(name="sein", bufs=4))
        accpool = ctx.enter_context(tc.tile_pool(name="acc", bufs=1))
        stpool = ctx.enter_context(tc.tile_pool(name="store", bufs=2))
        hkpool = ctx.enter_context(tc.tile_pool(name="hankel", bufs=2))
        hpool = ctx.enter_context(tc.tile_pool(name="acts", bufs=2))
        opool = ctx.enter_context(tc.tile_pool(name="outs", bufs=3))
        psmm = ctx.enter_context(tc.tile_pool(name="psmm", bufs=3, space="PSUM"))
        ps4p = ctx.enter_context(tc.tile_pool(name="ps4p", bufs=2, space="PSUM"))
        pscv = ctx.enter_context(tc.tile_pool(name="pscv", bufs=2, space="PSUM"))
        ptrp = ctx.enter_context(tc.tile_pool(name="ptrp", bufs=1, space="PSUM"))

        # ---- constants to SBUF (scalar queue: startup only) ----
        def wtile(dram, nm, m):
            t = cpool.tile([128, NKH, m], F8, name=nm, tag=nm)
            nc.scalar.dma_start(out=t[:, :, :], in_=dram[:, :])
            return t

        w1_sb = wtile(w1_h, "w1", HID_P)

        def emit_vh3(gi):
            t0g, blks, tot = groups[gi]
            t = hkpool.tile([128, NKH, GRP * BLK], F8, name="vh3", tag="vh3")
            dma = nc.gpsimd.dma_start(
                out=t[:, :, :tot],
                in_=bass.AP(v_h, t0g, [[1, 128], [128, NKH], [1, tot]]),
            )
            vh3_tiles[gi] = t
            return dma

        vh3_tiles = {}
        vh_dma0 = emit_vh3(0)

        w2_sb = wtile(w2_h, "w2", HID_P)
        w3_sb = wtile(w3_h, "w3", HID_P)
        w4_sb = wtile(w4_h, "w4", 32)

        sp = cpool.tile([128, 4 * SUB], BF, name="spk", tag="spk")
        nc.scalar.dma_start(out=sp[:, :], in_=spk_h[:, :])
        ek_sb = [sp[:, 0:SUB], sp[:, SUB : 2 * SUB]]
        ik_sb = [sp[:, 2 * SUB : 3 * SUB], sp[:, 3 * SUB : 4 * SUB]]
        k_syn = [(0, 128), (128, T_NO - 128)]

        bp = cpool.tile([128, 3 * NKH], FP32, name="biaspk", tag="biaspk")
        nc.scalar.dma_start(out=bp[:, :], in_=bpk_h[:, :])
        b4_sb = cpool.tile([SUB, 1], FP32, name="b4sb", tag="b4sb")
        b4_dma = nc.scalar.dma_start(
            out=b4_sb[:, 0:1], in_=bass.AP(b4_h, 0, [[1, SUB], [1, 1]]))
        ident = cpool.tile([128, 128], FP32, name="ident", tag="ident")
        make_identity(nc, ident[:, :])

        # ---- S_e/S_i reduction accumulators ----
        ACC_W = R_STEPS * RED_K
        se_acc = accpool.tile([128, ACC_W], FP32, name="se_acc", tag="se_acc")
        si_acc = accpool.tile([128, ACC_W], FP32, name="si_acc", tag="si_acc")

        reduced = 0  # reduce steps emitted so far
        stored = 0  # scratch columns stored so far
        EH = E_COLS // 2  # 1000
        IH = I_COLS // 2  # 250
        FSE = 704  # S_e column pairs folded on GpSimd; rest direct on DVE

        def emit_reduce(i):
            r0 = RED_K * 128 * i
            c = slice(RED_K * i, RED_K * (i + 1))
            # bulk loads ride the sync queue exclusively so their buffer
            # waits never block other DMA triggers
            se_t = sepool.tile([128, RED_K, 2, EH], F8, name="se_t", tag="se_t")
            first = nc.sync.dma_start(
                out=se_t[:, :, :, :],
                in_=bass.AP(se_h, r0 * E_COLS,
                            [[E_COLS, 128], [128 * E_COLS, RED_K], [1, E_COLS]]),
            )
            fe = sepool.tile([128, RED_K, FSE], BF, name="fold_e", tag="fold_e")
            nc.gpsimd.tensor_add(fe[:, :, :], se_t[:, :, 0, 0:FSE],
                                 se_t[:, :, 1, 0:FSE])
            rA = accpool.tile([128, RED_K], FP32, name="rA", tag="rA", bufs=2)
            nc.vector.reduce_sum(rA[:, :], fe[:, :, :], axis=mybir.AxisListType.X)
            rB = accpool.tile([128, RED_K], FP32, name="rB", tag="rB", bufs=2)
            nc.vector.reduce_sum(rB[:, :], se_t[:, :, :, FSE:EH],
                                 axis=mybir.AxisListType.XY)
            nc.vector.tensor_add(se_acc[:, c], rA[:, :], rB[:, :])
            si_t = sepool.tile([128, RED_K, 2, IH], F8, name="si_t", tag="si_t")
            nc.sync.dma_start(
                out=si_t[:, :, :, :],
                in_=bass.AP(si_h, r0 * I_COLS,
                            [[I_COLS, 128], [128 * I_COLS, RED_K], [1, I_COLS]]),
            )
            fi = sepool.tile([128, RED_K, IH], BF, name="fold_i", tag="fold_i")
            nc.gpsimd.tensor_add(fi[:, :, :], si_t[:, :, 0, :], si_t[:, :, 1, :])
            nc.vector.reduce_sum(si_acc[:, c], fi[:, :, :],
                                 axis=mybir.AxisListType.X)
            return first

        def emit_store(a, b):
            # PE-transpose fp32 accumulator cols [a,b) to [w,128], cast to
            # bf16, store contiguously to the scratch vector
            w = b - a
            for nm, acc, scr in (("se", se_acc, sse_h), ("si", si_acc, ssi_h)):
                tr_t = ptrp.tile([16, 128], FP32, name=f"{nm}tr", tag="tr")
                nc.tensor.transpose(tr_t[:w, :], acc[:, a:b], ident[:, :])
                st_t = stpool.tile([16, 128], BF, name=f"{nm}st", tag=f"{nm}st")
                nc.vector.tensor_copy(st_t[:w, :], tr_t[:w, :])
                nc.gpsimd.dma_start(
                    out=bass.AP(scr, 128 * a, [[128, w], [1, 128]]),
                    in_=st_t[:w, :],
                )

        def emit_stores_until(tgt_steps):
            nonlocal stored
            tgt = min(tgt_steps * RED_K, R_TILES)
            while stored < tgt:
                emit_store(stored, min(tgt, stored + 16))
                stored = min(tgt, stored + 16)

        def emit_reduces_until(tgt, gate=None):
            nonlocal reduced
            while reduced < tgt:
                first = emit_reduce(reduced)
                if reduced == 0 and gate is not None:
                    # let the startup-critical weight/hankel loads win the
                    # fabric before the bulk stream starts
                    add_dep_helper(first.ins, gate.ins, sync=True,
                                   reason="gate bulk stream on startup loads")
                reduced += 1

        synh_tiles = {}
        ff_tiles = {}

        def emit_synh(gi):
            t0g, blks, tot = groups[gi]
            synh = {}
            for nm, scr in (("se", sse_h), ("si", ssi_h)):
                t = hkpool.tile([128, GRP * BLK + SY_W], BF, name=f"{nm}h",
                                tag=f"{nm}h")
                nc.gpsimd.dma_start(
                    out=t[:, : tot + SY_W],
                    in_=bass.AP(scr, t0g, [[1, 128], [1, tot + SY_W]]),
                )
                synh[nm] = t
            synh_tiles[gi] = synh

        def emit_conv(gi):
            # conv matmuls + sum + tanh + store for a group whose ff and
            # scratch hankel operands were produced a group ago
            t0g, blks, tot = groups[gi]
            synh = synh_tiles.pop(gi)
            ffs = ff_tiles.pop(gi)
            for si_, (bt0, nt, coff) in enumerate(blks):
                psc = pscv.tile([SUB, BLK], FP32, name="psc", tag="psc")
                mm_i = 0
                for nm, k_sb in (("se", ek_sb), ("si", ik_sb)):
                    for j, (o, pk) in enumerate(k_syn):
                        nc.tensor.matmul(
                            psc[:, :nt], k_sb[j][:pk, :],
                            synh[nm][:pk, coff + 128 * j : coff + 128 * j + nt],
                            start=(mm_i == 0), stop=(mm_i == 3),
                        )
                        mm_i += 1
                t1 = opool.tile([SUB, BLK], FP32, name="t1", tag="t1")
                nc.vector.tensor_add(t1[:, :nt], ffs[si_][:, :nt],
                                     psc[0:SUB, :nt])
                out_sb = opool.tile([SUB, BLK], FP32, name="out_sb", tag="out_sb")
                nc.scalar.activation(out_sb[:, :nt], t1[:, :nt],
                                     mybir.ActivationFunctionType.Tanh,
                                     bias=b4_sb[:, 0:1], scale=1.0 / 16.0)
                nc.sync.dma_start(out=out_h[:, bt0 : bt0 + nt],
                                  in_=out_sb[:, :nt])

        def emit_vh3(gi):
            t0g, blks, tot = groups[gi]
            t = hkpool.tile([128, NKH, GRP * BLK], F8, name="vh3", tag="vh3")
            dma = nc.gpsimd.dma_start(
                out=t[:, :, :tot],
                in_=bass.AP(v_h, t0g, [[1, 128], [128, NKH], [1, tot]]),
            )
            vh3_tiles[gi] = t
            return dma

        vh3_tiles = {}
        vh_dma0 = emit_vh3(0)

        for gi, (t0g, blks, tot) in enumerate(groups):
            vh3 = vh3_tiles.pop(gi)
            if gi == 0:
                emit_reduces_until(needs[0], gate=vh_dma0)
            elif gi + 1 < NG:
                emit_vh3(gi + 1)

            halves = [blks[i : i + 2] for i in range(0, len(blks), 2)]

            # layers 1..3: fp8 DoubleRow, sub-pair interleaved.  The scratch
            # stores/hankel/conv for the PREVIOUS group are emitted between
            # L1 and L2: by then their reduce inputs are long since done, so
            # the PE-queue transposes never stall, and the conv executes
            # after this group's L4 with its hankel DMA well prefetched.
            h_prev = {}
            for lidx, w_sb in enumerate((w1_sb, w2_sb, w3_sb)):
                if lidx == 1:
                    if gi == 0 and NG > 1:
                        emit_vh3(1)
                    if gi >= 1:
                        emit_stores_until(needs[gi - 1])
                        emit_synh(gi - 1)
                h_cur = {}
                for half_i, half in enumerate(halves):
                    for mc in range(NKH):
                        pss = []
                        for si_, (bt0, nt, coff) in enumerate(half):
                            pss.append(psmm.tile([128, BLK], FP32, name="ps",
                                                 tag="ps"))
                        for j in range(2):
                            for si_, (bt0, nt, coff) in enumerate(half):
                                if lidx == 0:
                                    rhs = vh3[:, 2 * j : 2 * j + 2,
                                              coff : coff + nt]
                                else:
                                    hp = h_prev[(half_i, si_)]
                                    rhs = hp[:, 2 * j : 2 * j + 2, :nt]
                                nc.tensor.matmul(
                                    pss[si_][:, :nt],
                                    w_sb[:, 2 * j : 2 * j + 2,
                                         128 * mc : 128 * (mc + 1)],
                                    rhs,
                                    start=(j == 0), stop=(j == 1),
                                    perf_mode=mybir.MatmulPerfMode.DoubleRow,
                                )
                        for si_, (bt0, nt, coff) in enumerate(half):
                            if mc == 0:
                                h_cur[(half_i, si_)] = hpool.tile(
                                    [128, NKH, BLK], F8,
                                    name=f"h{lidx}", tag=f"h{lidx}_{half_i}{si_}")
                            nc.scalar.activation(
                                h_cur[(half_i, si_)][:, mc, :nt],
                                pss[si_][:, :nt],
                                mybir.ActivationFunctionType.Lrelu,
                                bias=bp[:, lidx * NKH + mc : lidx * NKH + mc + 1],
                                scale=1.0 / WS, alpha=0.01,
                            )
                h_prev = h_cur

            # layer 4 feed-forward part: ff = (W4 h3)  (b4 applied at tanh)
            ffs = []
            for half_i, half in enumerate(halves):
                for si_, (bt0, nt, coff) in enumerate(half):
                    h3 = h_prev[(half_i, si_)]
                    ps4 = ps4p.tile([32, BLK], FP32, name="ps4", tag="ps4")
                    for j in range(2):
                        nc.tensor.matmul(
                            ps4[:, :nt], w4_sb[:, 2 * j : 2 * j + 2, :],
                            h3[:, 2 * j : 2 * j + 2, :nt],
                            start=(j == 0), stop=(j == 1),
                            perf_mode=mybir.MatmulPerfMode.DoubleRow,
                        )
                    ff_t = opool.tile([SUB, BLK], FP32, name="ff_sb", tag="ff_sb",
                                      bufs=12)
                    nc.vector.tensor_copy(ff_t[:, :nt], ps4[0:SUB, :nt])
                    ffs.append(ff_t)
            ff_tiles[gi] = ffs

            if gi >= 2:
                emit_conv(gi - 2)

            # scratch stores for THIS group's window: the PE transposes sit
            # behind all of this group's matmuls, by which time the reduce
            # stream is long past them; the conv consumes them a group later
            emit_stores_until(needs[gi])
            emit_synh(gi)

            # bulk reduce loads one group ahead
            emit_reduces_until(needs[min(gi + 1, NG - 1)])

        emit_conv(NG - 2)
        emit_conv(NG - 1)

    nc.compile()
    return nc


def _fast_fp8(x):
    return np.asarray(x, np.float32).astype(E4M3)


def kernel(V, S_e, S_i, W1, b1, W2, b2, W3, b3, W4, b4, W_syn, Tau_syn, Delta_syn):
    V = np.asarray(V, np.float32)
    T = V.shape[0]
    assert T % N_CORES == 0
    T_LOC = T // N_CORES
    T_PAD = _ceil_div(T_LOC, 128) * 128
    SE_ROWS = T_NO - 1 + T_LOC
    R_STEPS = _ceil_div(_ceil_div(SE_ROWS, 128), RED_K)
    R_ROWS = R_STEPS * RED_K * 128
    V_LEN = T_PAD + WIN - 1 + 128
    HID = W1.shape[0]
    SUB = W4.shape[0]
    E_COLS = S_e.shape[1]
    I_COLS = S_i.shape[1]

    # ---- host-side prep (layout/dtype only + 20x200 conv kernels) ----
    W_syn = np.asarray(W_syn, np.float32)
    Tau_syn = np.asarray(Tau_syn, np.float32)
    Delta_syn = np.asarray(Delta_syn, np.float32)
    t_raw = np.arange(T_NO, dtype=np.float32)[None, :]
    t_e = np.maximum(t_raw - Delta_syn[:, 0:1], 0.0)
    t_i = np.maximum(t_raw - Delta_syn[:, 1:2], 0.0)
    tt_e = t_e / Tau_syn[:, 0:1] ** 2
    tt_i = t_i / Tau_syn[:, 1:2] ** 2
    e_kern = tt_e * np.exp(-tt_e) * W_syn[:, 0:1] ** 2
    i_kern = -(tt_i * np.exp(-tt_i)) * W_syn[:, 1:2] ** 2
    ekm = np.ascontiguousarray(e_kern[:, ::-1].T)  # [T_NO, SUB]
    ikm = np.ascontiguousarray(i_kern[:, ::-1].T)
    spk = np.zeros((128, 4 * SUB), np.float32)
    spk[0:128, 0:SUB] = ekm[0:128]
    spk[0 : T_NO - 128, SUB : 2 * SUB] = ekm[128:T_NO]
    spk[0:128, 2 * SUB : 3 * SUB] = ikm[0:128]
    spk[0 : T_NO - 128, 3 * SUB : 4 * SUB] = ikm[128:T_NO]

    def pack_w(Wm, m_pad):
        # [fan_out, K] -> [128, 4, m_pad] fp8: [p, kc, m] = WS*W[m, 128kc+p]
        k = Wm.shape[1]
        out = np.zeros((128, 4, m_pad), np.float32)
        wt = np.asarray(Wm, np.float32).T * WS  # [K, fan_out]
        for kc in range(4):
            r0, r1 = 128 * kc, min(k, 128 * (kc + 1))
            if r0 < k:
                out[: r1 - r0, kc, : Wm.shape[0]] = wt[r0:r1]
        return _fast_fp8(out).reshape(128, -1)

    wd = {
        "w1": pack_w(W1, 512),
        "w2": pack_w(W2, 512),
        "w3": pack_w(W3, 512),
        "w4": pack_w(W4, 32),
        "bpk": np.concatenate(
            [np.pad(np.asarray(b, np.float32)[:, None], ((0, 12), (0, 0)))
             .reshape(128, 4, order="F") for b in (b1, b2, b3)], 1
        ).astype(np.float32),
        "b4": np.asarray(b4, np.float32),
        "spk": spk.astype(BF16),
    }

    vg = np.zeros(T_NO - 1 + T + V_LEN - T_LOC, np.float32)
    vg[T_NO - 1 : T_NO - 1 + T] = V
    vg = _fast_fp8(vg)

    S_e8 = _fast_fp8(S_e)
    S_i8 = _fast_fp8(S_i)
    halo = T_NO - 1
    pad_rows = R_ROWS - SE_ROWS
    in_maps = []
    for m in range(N_CORES):
        r0 = m * T_LOC
        a = r0 - halo
        se_m = np.zeros((R_ROWS, E_COLS), E4M3)
        si_m = np.zeros((R_ROWS, I_COLS), E4M3)
        lo = max(0, a)
        se_m[lo - a : lo - a + min(SE_ROWS - (lo - a), T - lo)] = \
            S_e8[lo : min(a + SE_ROWS, T)]
        si_m[lo - a : lo - a + min(SE_ROWS - (lo - a), T - lo)] = \
            S_i8[lo : min(a + SE_ROWS, T)]
        in_maps.append(
            {"se": se_m, "si": si_m, "v": vg[r0 : r0 + V_LEN], **wd}
        )

    nc = _build(T_PAD, SE_ROWS, E_COLS, I_COLS, HID, SUB)
    trace = os.environ.get("CC_TRACE") == "1"
    res = run_bass_kernel_spmd(nc, in_maps, list(range(N_CORES)), trace=trace)
    LAST["exec_time_ns"] = res.exec_time_ns
    LAST["results"] = res
    out = np.concatenate(
        [res.results[m]["out"][:, :T_LOC].T for m in range(N_CORES)], 0
    )
    return np.ascontiguousarray(out.astype(np.float32))


# revision 48
# speedup vs baseline: 1.0989x; 1.0989x over previous
"""Trainium2 Bass kernel for nn_Encoder (sliding-window MLP + synaptic conv).

Computation (per timestep t of T_data):
  syn_e[t] = sum(S_e[t, :]);  syn_i[t] = sum(S_i[t, :])
  syn_out[t, s] = sum_k e_kern[s, k] * syn_e[t-k] + i_kern[s, k] * syn_i[t-k]
  Vw[t, :] = V[t-199 : t+200]   (zero padded)
  h = lrelu(Vw @ W1.T + b1); h = lrelu(h @ W2.T + b2); h = lrelu(h @ W3.T + b3)
  out[t, :] = tanh(h @ W4.T + b4 + syn_out[t, :])

Strategy: data-parallel over T across 8 NeuronCores; each core gets its T/8
slice plus a 199-row halo of S_e/S_i and a V halo (host zero-pads edges).
Per core:
  - S_e/S_i stream in fp8e4 (4 row-tiles per DMA into [128,4,C] tiles),
    GpSimd folds column halves fp8->bf16, VectorE free-axis reduces the
    bf16 fold at 2x rate into fp32 accumulators; accumulators are
    PE-transposed and stored to a DRAM scratch vector in bf16.
  - MLP in fp8e4 with DoubleRow matmuls (2 K-chunks of 128 per PE
    instruction): weights live as [128, 4, 512] tiles (16x scaled),
    activations as [128, 4, 512] fp8 tiles whose planes are hid chunks.
    The V sliding window is materialized as a [128, 4, W] Hankel tile via
    a single 3D overlapped DMA (plane kc = window offset 128*kc).
  - Blocks of 512 timesteps are processed in groups of 4, sub-paired
    (a,b)/(c,d): each DoubleRow stationary serves 2 consecutive matmuls
    (hides the slow DR LDWEIGHTS) and each layer's PSUM evictions retire
    under the other sub-pair's matmuls.
  - ScalarE evicts PSUM with fused lrelu (scale=1/16 undoes the weight
    scaling, bias folded in); layer-4 ff evicts via VectorE copy.
  - Conv runs one group behind the MLP (scratch operands guaranteed
    ready): e/i kernels matmul into col-grouped PSUM [0:20]/[32:52] of
    one bank, then ff + e + i are summed by 2 VectorE adds and ScalarE
    applies tanh with b4 as the per-partition activation bias.
"""

import os
from contextlib import ExitStack

import ml_dtypes
import numpy as np

import concourse.bass as bass
import concourse.mybir as mybir
import concourse.tile as tile
from concourse import bacc
from concourse.bass_utils import run_bass_kernel_spmd
from concourse.masks import make_identity
from concourse.tile_rust import add_dep_helper

BF16 = ml_dtypes.bfloat16
E4M3 = ml_dtypes.float8_e4m3fn
FP32 = mybir.dt.float32
BF = mybir.dt.bfloat16
F8 = mybir.dt.float8e4

T_NO = 200
WIN = 2 * T_NO - 1  # 399
N_CORES = 8
BLK = 512  # timesteps per block (one PSUM bank of fp32)
GRP = 4  # blocks per group
RED_K = 4  # S_e/S_i row-tiles per DMA/fold/reduce step
WS = 16.0  # fp8 weight scale

LAST = {}  # exec_time_ns / trace info from the most recent run (for test harness)


def _ceil_div(a, b):
    return -(-a // b)


def _build(T_PAD, SE_ROWS, E_COLS, I_COLS, HID, SUB):
    """Build the per-core Bass program (identical on all 8 cores)."""
    R_TILES = _ceil_div(SE_ROWS, 128)
    R_STEPS = _ceil_div(R_TILES, RED_K)
    R_ROWS = R_STEPS * RED_K * 128  # host pads S_e/S_i to this many rows
    SCR_LEN = R_STEPS * RED_K * 128 + 128
    V_LEN = T_PAD + WIN - 1 + 128
    NB = _ceil_div(T_PAD, BLK)
    NKH = 4  # K chunks of both WIN (399) and HID (500), 128-padded
    HID_P = 512

    nc = bacc.Bacc(
        "TRN2", target_bir_lowering=False, debug=False, num_devices=N_CORES
    )

    se_h = nc.dram_tensor("se", [R_ROWS, E_COLS], F8, kind="ExternalInput")
    si_h = nc.dram_tensor("si", [R_ROWS, I_COLS], F8, kind="ExternalInput")
    v_h = nc.dram_tensor("v", [V_LEN], F8, kind="ExternalInput")
    w1_h = nc.dram_tensor("w1", [128, NKH * HID_P], F8, kind="ExternalInput")
    w2_h = nc.dram_tensor("w2", [128, NKH * HID_P], F8, kind="ExternalInput")
    w3_h = nc.dram_tensor("w3", [128, NKH * HID_P], F8, kind="ExternalInput")
    w4_h = nc.dram_tensor("w4", [128, NKH * 32], F8, kind="ExternalInput")
    bpk_h = nc.dram_tensor("bpk", [128, 3 * NKH], FP32, kind="ExternalInput")
    b4_h = nc.dram_tensor("b4", [SUB], FP32, kind="ExternalInput")
    spk_h = nc.dram_tensor("spk", [128, 4 * SUB], BF, kind="ExternalInput")
    out_h = nc.dram_tensor("out", [SUB, T_PAD], FP32, kind="ExternalOutput")

    sse_h = nc.dram_tensor("sse_scratch", [SCR_LEN], BF)
    ssi_h = nc.dram_tensor("ssi_scratch", [SCR_LEN], BF)

    # group structure: a short first group (2 blocks) so its scratch window
    # is ready early, then groups of 4
    blocks = []
    for b in range(NB):
        blocks.append((BLK * b, min(BLK, T_PAD - BLK * b)))
    starts = [0] + list(range(min(2, NB), NB, GRP))
    groups = []
    for si in range(len(starts)):
        g0 = starts[si]
        g1 = starts[si + 1] if si + 1 < len(starts) else NB
        blks = []
        off = 0
        for b in range(g0, g1):
            bt0, nt = blocks[b]
            blks.append((bt0, nt, off))
            off += nt
        groups.append((BLK * g0, blks, off))
    NG = len(groups)
    SY_W = 128  # extra hankel cols for the 2nd conv K chunk

    # reduce steps needed before the conv of group g can load its hankel
    needs = []
    for gi, (t0g, blks, tot) in enumerate(groups):
        if gi == NG - 1:
            needs.append(R_STEPS)
        else:
            needs.append(min(R_STEPS,
                             _ceil_div(_ceil_div(t0g + tot + T_NO - 1, 128),
                                       RED_K)))

    with tile.TileContext(nc) as tc, ExitStack() as ctx:
        cpool = ctx.enter_context(tc.tile_pool(name="consts", bufs=1))
        sepool = ctx.enter_context(tc.tile_pool(name="sein", bufs=4))
        accpool = ctx.enter_context(tc.tile_pool(name="acc", bufs=1))
        stpool = ctx.enter_context(tc.tile_pool(name="store", bufs=2))
        hkpool = ctx.enter_context(tc.tile_pool(name="hankel", bufs=2))
        hpool = ctx.enter_context(tc.tile_pool(name="acts", bufs=2))
        opool = ctx.enter_context(tc.tile_pool(name="outs", bufs=3))
        psmm = ctx.enter_context(tc.tile_pool(name="psmm", bufs=3, space="PSUM"))
        ps4p = ctx.enter_context(tc.tile_pool(name="ps4p", bufs=2, space="PSUM"))
        pscv = ctx.enter_context(tc.tile_pool(name="pscv", bufs=2, space="PSUM"))
        ptrp = ctx.enter_context(tc.tile_pool(name="ptrp", bufs=1, space="PSUM"))

        # ---- constants to SBUF (scalar queue: startup only) ----
        def wtile(dram, nm, m):
            t = cpool.tile([128, NKH, m], F8, name=nm, tag=nm)
            nc.scalar.dma_start(out=t[:, :, :], in_=dram[:, :])
            return t

        w1_sb = wtile(w1_h, "w1", HID_P)

        def emit_vh3(gi):
            t0g, blks, tot = groups[gi]
            t = hkpool.tile([128, NKH, GRP * BLK], F8, name="vh3", tag="vh3")
            dma = nc.scalar.dma_start(
                out=t[:, :, :tot],
                in_=bass.AP(v_h, t0g, [[1, 128], [128, NKH], [1, tot]]),
            )
            vh3_tiles[gi] = t
            return dma

        vh3_tiles = {}
        vh_dma0 = emit_vh3(0)

        w2_sb = wtile(w2_h, "w2", HID_P)
        w3_sb = wtile(w3_h, "w3", HID_P)
        w4_sb = wtile(w4_h, "w4", 32)

        sp = cpool.tile([128, 4 * SUB], BF, name="spk", tag="spk")
        nc.scalar.dma_start(out=sp[:, :], in_=spk_h[:, :])
        ek_sb = [sp[:, 0:SUB], sp[:, SUB : 2 * SUB]]
        ik_sb = [sp[:, 2 * SUB : 3 * SUB], sp[:, 3 * SUB : 4 * SUB]]
        k_syn = [(0, 128), (128, T_NO - 128)]

        bp = cpool.tile([128, 3 * NKH], FP32, name="biaspk", tag="biaspk")
        nc.scalar.dma_start(out=bp[:, :], in_=bpk_h[:, :])
        b4_sb = cpool.tile([SUB, 1], FP32, name="b4sb", tag="b4sb")
        b4_dma = nc.scalar.dma_start(
            out=b4_sb[:, 0:1], in_=bass.AP(b4_h, 0, [[1, SUB], [1, 1]]))
        ident = cpool.tile([128, 128], FP32, name="ident", tag="ident")
        make_identity(nc, ident[:, :])

        # ---- S_e/S_i reduction accumulators ----
        ACC_W = R_STEPS * RED_K
        se_acc = accpool.tile([128, ACC_W], FP32, name="se_acc", tag="se_acc")
        si_acc = accpool.tile([128, ACC_W], FP32, name="si_acc", tag="si_acc")

        reduced = 0  # reduce steps emitted so far
        stored = 0  # scratch columns stored so far
        EH = E_COLS // 2  # 1000
        IH = I_COLS // 2  # 250
        FSE = 704  # S_e column pairs folded on GpSimd; rest direct on DVE

        def emit_reduce(i):
            r0 = RED_K * 128 * i
            c = slice(RED_K * i, RED_K * (i + 1))
            # bulk loads ride the sync queue exclusively so their buffer
            # waits never block other DMA triggers
            se_t = sepool.tile([128, RED_K, 2, EH], F8, name="se_t", tag="se_t")
            first = nc.sync.dma_start(
                out=se_t[:, :, :, :],
                in_=bass.AP(se_h, r0 * E_COLS,
                            [[E_COLS, 128], [128 * E_COLS, RED_K], [1, E_COLS]]),
            )
            fe = sepool.tile([128, RED_K, FSE], BF, name="fold_e", tag="fold_e")
            nc.gpsimd.tensor_add(fe[:, :, :], se_t[:, :, 0, 0:FSE],
                                 se_t[:, :, 1, 0:FSE])
            rA = accpool.tile([128, RED_K], FP32, name="rA", tag="rA", bufs=2)
            nc.vector.reduce_sum(rA[:, :], fe[:, :, :], axis=mybir.AxisListType.X)
            rB = accpool.tile([128, RED_K], FP32, name="rB", tag="rB", bufs=2)
            nc.vector.reduce_sum(rB[:, :], se_t[:, :, :, FSE:EH],
                                 axis=mybir.AxisListType.XY)
            nc.vector.tensor_add(se_acc[:, c], rA[:, :], rB[:, :])
            si_t = sepool.tile([128, RED_K, 2, IH], F8, name="si_t", tag="si_t")
            nc.sync.dma_start(
                out=si_t[:, :, :, :],
                in_=bass.AP(si_h, r0 * I_COLS,
                            [[I_COLS, 128], [128 * I_COLS, RED_K], [1, I_COLS]]),
            )
            fi = sepool.tile([128, RED_K, IH], BF, name="fold_i", tag="fold_i")
            nc.gpsimd.tensor_add(fi[:, :, :], si_t[:, :, 0, :], si_t[:, :, 1, :])
            nc.vector.reduce_sum(si_acc[:, c], fi[:, :, :],
                                 axis=mybir.AxisListType.X)
            return first

        def emit_store(a, b):
            # PE-transpose fp32 accumulator cols [a,b) to [w,128], cast to
            # bf16, store contiguously to the scratch vector
            w = b - a
            for nm, acc, scr in (("se", se_acc, sse_h), ("si", si_acc, ssi_h)):
                tr_t = ptrp.tile([16, 128], FP32, name=f"{nm}tr", tag="tr")
                nc.tensor.transpose(tr_t[:w, :], acc[:, a:b], ident[:, :])
                st_t = stpool.tile([16, 128], BF, name=f"{nm}st", tag=f"{nm}st")
                nc.vector.tensor_copy(st_t[:w, :], tr_t[:w, :])
                nc.gpsimd.dma_start(
                    out=bass.AP(scr, 128 * a, [[128, w], [1, 128]]),
                    in_=st_t[:w, :],
                )

        def emit_stores_until(tgt_steps):
            nonlocal stored
            tgt = min(tgt_steps * RED_K, R_TILES)
            while stored < tgt:
                emit_store(stored, min(tgt, stored + 16))
                stored = min(tgt, stored + 16)

        def emit_reduces_until(tgt, gate=None):
            nonlocal reduced
            while reduced < tgt:
                first = emit_reduce(reduced)
                if reduced == 0 and gate is not None:
                    # let the startup-critical weight/hankel loads win the
                    # fabric before the bulk stream starts
                    add_dep_helper(first.ins, gate.ins, sync=True,
                                   reason="gate bulk stream on startup loads")
                reduced += 1

        synh_tiles = {}
        ff_tiles = {}

        def emit_synh(gi):
            t0g, blks, tot = groups[gi]
            synh = {}
            for nm, scr in (("se", sse_h), ("si", ssi_h)):
                t = hkpool.tile([128, GRP * BLK + SY_W], BF, name=f"{nm}h",
                                tag=f"{nm}h", bufs=3)
                nc.gpsimd.dma_start(
                    out=t[:, : tot + SY_W],
                    in_=bass.AP(scr, t0g, [[1, 128], [1, tot + SY_W]]),
                )
                synh[nm] = t
            synh_tiles[gi] = synh

        def emit_conv(gi):
            # conv matmuls + sum + tanh + store for a group whose ff and
            # scratch hankel operands were produced a group ago
            t0g, blks, tot = groups[gi]
            synh = synh_tiles.pop(gi)
            ffs = ff_tiles.pop(gi)
            for si_, (bt0, nt, coff) in enumerate(blks):
                psc = pscv.tile([SUB, BLK], FP32, name="psc", tag="psc")
                mm_i = 0
                for nm, k_sb in (("se", ek_sb), ("si", ik_sb)):
                    for j, (o, pk) in enumerate(k_syn):
                        nc.tensor.matmul(
                            psc[:, :nt], k_sb[j][:pk, :],
                            synh[nm][:pk, coff + 128 * j : coff + 128 * j + nt],
                            start=(mm_i == 0), stop=(mm_i == 3),
                        )
                        mm_i += 1
                t1 = opool.tile([SUB, BLK], FP32, name="t1", tag="t1")
                nc.vector.tensor_add(t1[:, :nt], ffs[si_][:, :nt],
                                     psc[0:SUB, :nt])
                out_sb = opool.tile([SUB, BLK], FP32, name="out_sb", tag="out_sb")
                nc.scalar.activation(out_sb[:, :nt], t1[:, :nt],
                                     mybir.ActivationFunctionType.Tanh,
                                     bias=b4_sb[:, 0:1], scale=1.0 / 16.0)
                nc.sync.dma_start(out=out_h[:, bt0 : bt0 + nt],
                                  in_=out_sb[:, :nt])

        def emit_vh3(gi):
            t0g, blks, tot = groups[gi]
            t = hkpool.tile([128, NKH, GRP * BLK], F8, name="vh3", tag="vh3")
            dma = nc.scalar.dma_start(
                out=t[:, :, :tot],
                in_=bass.AP(v_h, t0g, [[1, 128], [128, NKH], [1, tot]]),
            )
            vh3_tiles[gi] = t
            return dma

        vh3_tiles = {}
        vh_dma0 = emit_vh3(0)

        for gi, (t0g, blks, tot) in enumerate(groups):
            vh3 = vh3_tiles.pop(gi)
            if gi == 0:
                emit_reduces_until(needs[0], gate=vh_dma0)
            elif gi + 1 < NG:
                emit_vh3(gi + 1)

            halves = [blks[i : i + 2] for i in range(0, len(blks), 2)]

            # layers 1..3: fp8 DoubleRow, sub-pair interleaved.  The scratch
            # stores/hankel/conv for the PREVIOUS group are emitted between
            # L1 and L2: by then their reduce inputs are long since done, so
            # the PE-queue transposes never stall, and the conv executes
            # after this group's L4 with its hankel DMA well prefetched.
            h_prev = {}
            for lidx, w_sb in enumerate((w1_sb, w2_sb, w3_sb)):
                if lidx == 1:
                    if gi == 0 and NG > 1:
                        emit_vh3(1)
                    if gi >= 1:
                        emit_stores_until(needs[gi - 1])
                        emit_synh(gi - 1)
                h_cur = {}
                for half_i, half in enumerate(halves):
                    for mc in range(NKH):
                        pss = []
                        for si_, (bt0, nt, coff) in enumerate(half):
                            pss.append(psmm.tile([128, BLK], FP32, name="ps",
                                                 tag="ps"))
                        for j in range(2):
                            for si_, (bt0, nt, coff) in enumerate(half):
                                if lidx == 0:
                                    rhs = vh3[:, 2 * j : 2 * j + 2,
                                              coff : coff + nt]
                                else:
                                    hp = h_prev[(half_i, si_)]
                                    rhs = hp[:, 2 * j : 2 * j + 2, :nt]
                                nc.tensor.matmul(
                                    pss[si_][:, :nt],
                                    w_sb[:, 2 * j : 2 * j + 2,
                                         128 * mc : 128 * (mc + 1)],
                                    rhs,
                                    start=(j == 0), stop=(j == 1),
                                    perf_mode=mybir.MatmulPerfMode.DoubleRow,
                                )
                        for si_, (bt0, nt, coff) in enumerate(half):
                            if mc == 0:
                                h_cur[(half_i, si_)] = hpool.tile(
                                    [128, NKH, BLK], F8,
                                    name=f"h{lidx}", tag=f"h{lidx}_{half_i}{si_}")
                            nc.scalar.activation(
                                h_cur[(half_i, si_)][:, mc, :nt],
                                pss[si_][:, :nt],
                                mybir.ActivationFunctionType.Lrelu,
                                bias=bp[:, lidx * NKH + mc : lidx * NKH + mc + 1],
                                scale=1.0 / WS, alpha=0.01,
                            )
                h_prev = h_cur

            # layer 4 feed-forward part: ff = (W4 h3)  (b4 applied at tanh)
            ffs = []
            for half_i, half in enumerate(halves):
                for si_, (bt0, nt, coff) in enumerate(half):
                    h3 = h_prev[(half_i, si_)]
                    ps4 = ps4p.tile([32, BLK], FP32, name="ps4", tag="ps4")
                    for j in range(2):
                        nc.tensor.matmul(
                            ps4[:, :nt], w4_sb[:, 2 * j : 2 * j + 2, :],
                            h3[:, 2 * j : 2 * j + 2, :nt],
                            start=(j == 0), stop=(j == 1),
                            perf_mode=mybir.MatmulPerfMode.DoubleRow,
                        )
                    ff_t = opool.tile([SUB, BLK], FP32, name="ff_sb", tag="ff_sb",
                                      bufs=12)
                    nc.vector.tensor_copy(ff_t[:, :nt], ps4[0:SUB, :nt])
                    ffs.append(ff_t)
            ff_tiles[gi] = ffs

            if gi >= 2:
                emit_conv(gi - 2)

            # scratch stores for THIS group's window: the PE transposes sit
            # behind all of this group's matmuls, by which time the reduce
            # stream is long past them; the conv consumes them a group later
            emit_stores_until(needs[gi])
            emit_synh(gi)

            # bulk reduce loads one group ahead
            emit_reduces_until(needs[min(gi + 1, NG - 1)])

        emit_conv(NG - 2)
        emit_conv(NG - 1)

    nc.compile()
    return nc


def _fast_fp8(x):
    return np.asarray(x, np.float32).astype(E4M3)


def kernel(V, S_e, S_i, W1, b1, W2, b2, W3, b3, W4, b4, W_syn, Tau_syn, Delta_syn):
    V = np.asarray(V, np.float32)
    T = V.shape[0]
    assert T % N_CORES == 0
    T_LOC = T // N_CORES
    T_PAD = _ceil_div(T_LOC, 128) * 128
    SE_ROWS = T_NO - 1 + T_LOC
    R_STEPS = _ceil_div(_ceil_div(SE_ROWS, 128), RED_K)
    R_ROWS = R_STEPS * RED_K * 128
    V_LEN = T_PAD + WIN - 1 + 128
    HID = W1.shape[0]
    SUB = W4.shape[0]
    E_COLS = S_e.shape[1]
    I_COLS = S_i.shape[1]

    # ---- host-side prep (layout/dtype only + 20x200 conv kernels) ----
    W_syn = np.asarray(W_syn, np.float32)
    Tau_syn = np.asarray(Tau_syn, np.float32)
    Delta_syn = np.asarray(Delta_syn, np.float32)
    t_raw = np.arange(T_NO, dtype=np.float32)[None, :]
    t_e = np.maximum(t_raw - Delta_syn[:, 0:1], 0.0)
    t_i = np.maximum(t_raw - Delta_syn[:, 1:2], 0.0)
    tt_e = t_e / Tau_syn[:, 0:1] ** 2
    tt_i = t_i / Tau_syn[:, 1:2] ** 2
    e_kern = tt_e * np.exp(-tt_e) * W_syn[:, 0:1] ** 2
    i_kern = -(tt_i * np.exp(-tt_i)) * W_syn[:, 1:2] ** 2
    ekm = np.ascontiguousarray(e_kern[:, ::-1].T)  # [T_NO, SUB]
    ikm = np.ascontiguousarray(i_kern[:, ::-1].T)
    spk = np.zeros((128, 4 * SUB), np.float32)
    spk[0:128, 0:SUB] = ekm[0:128]
    spk[0 : T_NO - 128, SUB : 2 * SUB] = ekm[128:T_NO]
    spk[0:128, 2 * SUB : 3 * SUB] = ikm[0:128]
    spk[0 : T_NO - 128, 3 * SUB : 4 * SUB] = ikm[128:T_NO]

    def pack_w(Wm, m_pad):
        # [fan_out, K] -> [128, 4, m_pad] fp8: [p, kc, m] = WS*W[m, 128kc+p]
        k = Wm.shape[1]
        out = np.zeros((128, 4, m_pad), np.float32)
        wt = np.asarray(Wm, np.float32).T * WS  # [K, fan_out]
        for kc in range(4):
            r0, r1 = 128 * kc, min(k, 128 * (kc + 1))
            if r0 < k:
                out[: r1 - r0, kc, : Wm.shape[0]] = wt[r0:r1]
        return _fast_fp8(out).reshape(128, -1)

    wd = {
        "w1": pack_w(W1, 512),
        "w2": pack_w(W2, 512),
        "w3": pack_w(W3, 512),
        "w4": pack_w(W4, 32),
        "bpk": np.concatenate(
            [np.pad(np.asarray(b, np.float32)[:, None], ((0, 12), (0, 0)))
             .reshape(128, 4, order="F") for b in (b1, b2, b3)], 1
        ).astype(np.float32),
        "b4": np.asarray(b4, np.float32),
        "spk": spk.astype(BF16),
    }

    vg = np.zeros(T_NO - 1 + T + V_LEN - T_LOC, np.float32)
    vg[T_NO - 1 : T_NO - 1 + T] = V
    vg = _fast_fp8(vg)

    S_e8 = _fast_fp8(S_e)
    S_i8 = _fast_fp8(S_i)
    halo = T_NO - 1
    pad_rows = R_ROWS - SE_ROWS
    in_maps = []
    for m in range(N_CORES):
        r0 = m * T_LOC
        a = r0 - halo
        se_m = np.zeros((R_ROWS, E_COLS), E4M3)
        si_m = np.zeros((R_ROWS, I_COLS), E4M3)
        lo = max(0, a)
        se_m[lo - a : lo - a + min(SE_ROWS - (lo - a), T - lo)] = \
            S_e8[lo : min(a + SE_ROWS, T)]
        si_m[lo - a : lo - a + min(SE_ROWS - (lo - a), T - lo)] = \
            S_i8[lo : min(a + SE_ROWS, T)]
        in_maps.append(
            {"se": se_m, "si": si_m, "v": vg[r0 : r0 + V_LEN], **wd}
        )

    nc = _build(T_PAD, SE_ROWS, E_COLS, I_COLS, HID, SUB)
    trace = os.environ.get("CC_TRACE") == "1"
    res = run_bass_kernel_spmd(nc, in_maps, list(range(N_CORES)), trace=trace)
    LAST["exec_time_ns"] = res.exec_time_ns
    LAST["results"] = res
    out = np.concatenate(
        [res.results[m]["out"][:, :T_LOC].T for m in range(N_CORES)], 0
    )
    return np.ascontiguousarray(out.astype(np.float32))
